# revision 37
# baseline (speedup 1.0000x reference)
"""DigitCaps dynamic-routing kernel for Trainium2 (8 NeuronCores, SPMD).

Problem:  u = einsum('bri,rcio->brco', x, W[0]);  3 routing iterations
          (softmax over capsules, weighted sum over routes, squash,
          agreement update);  returns v [B, C, OC].

Shapes: B=256, R=1152, C=10, IC=8, OC=16.  Batch-sharded 8 ways (BL=32
per core, zero cross-core communication).

Design notes (per core):
 - u-phase: r is processed in 72 chunks of G=16 routes.  For each chunk,
   lhsT is a block-diagonal arrangement of x ([128=(g,i), 128=(b8,g16)]
   per b-group of 8) so a single matmul produces u for 8 batches x 16
   routes = 128 PSUM partitions; rhs is the natural W chunk
   [128=(g,i), 160=(c,o)].  bf16 operands, fp32 PSUM accumulate.
 - u stays resident in SBUF as bf16 [p=(b8,g16), f=(bg4, k72, c10, o16)].
 - Iteration 1 shortcut: c is uniform 1/10, so s = 0.1*sum_r u, computed
   for free during the u-phase by one extra accumulating matmul per chunk
   with plain (non-block-diag) x as lhsT.
 - b-update (delta = sum_o u*v) runs on the vector engine (mul at 2x +
   strided reduce at 1x: this is the kernel's port-bound critical path);
   softmax on scalar+vector engines (max-subtraction skipped: logits are
   O(30), fp32 exp is safe and matches jax softmax up to rounding).
 - s-pass (s = sum_r c*u) runs on the tensor engine: lhsT is block-diag
   c [(b,g), (c10,b'8)] built by a gpsimd mask-multiply, rhs is the
   resident u slice, PSUM-accumulated over all 72 chunks; the (c,c')
   diagonal is then extracted with small DMAs (engine APs cannot express
   partition-crossing diagonals, DMAs can move between partition bases).
 - Everything in the routing iterations is processed per 8-batch group
   (and per 36-chunk half) so the DVE chain of one group overlaps the
   gpsimd/PE/DMA tail of the previous one, including across iterations
   (per-group squash + v-broadcast feeds the next iteration early).
"""

import sys

sys.path.insert(0, "/opt/trn_rl_repo")

from contextlib import ExitStack

import ml_dtypes
import numpy as np

import concourse.bass as bass
import concourse.tile as tile
from concourse import bacc, mybir
from concourse.bass_utils import run_bass_kernel_spmd

BF16 = mybir.dt.bfloat16
F32 = mybir.dt.float32
AF = mybir.ActivationFunctionType
ALU = mybir.AluOpType
AX = mybir.AxisListType

B, R, C, IC, OC = 256, 1152, 10, 8, 16
NCORES = 8
BL = B // NCORES  # 32 batches per core
G = 16  # routes per chunk
NBG = BL // 8  # 4 b-groups of 8
CO = C * OC  # 160
EPS = 1e-8
NPBF = ml_dtypes.bfloat16

# Set by tests to shrink the problem for simulation; full size by default.
_R_OVERRIDE = None


def _nchunks(r=None):
    r = r if r is not None else (_R_OVERRIDE or R)
    assert r % G == 0
    return r // G


def _squash(nc, pool, s_ap, v_ap, pre, np_=8):
    """v = squash(pre*s) for s,v [np_, CO] fp32 SBUF tiles (base partition 0).

    Uses sqrt(n2 + EPS^2) ~= nrm + EPS (difference is O(EPS) absolute,
    only relevant when the norm itself is ~EPS).
    """
    sq = pool.tile([np_, CO], F32, tag="sq")
    if pre == 1.0:
        nc.vector.tensor_tensor(out=sq[:], in0=s_ap, in1=s_ap, op=ALU.mult)
    else:
        nc.scalar.activation(sq[:], s_ap, AF.Square, scale=pre)  # (pre*s)^2
    n2 = pool.tile([np_, C], F32, tag="n2")
    nc.vector.reduce_sum(
        out=n2[:], in_=sq[:].rearrange("p (c o) -> p c o", c=C), axis=AX.X
    )
    nrm = pool.tile([np_, C], F32, tag="nrm")
    nc.scalar.sqrt(nrm[:], n2[:])
    t1 = pool.tile([np_, C], F32, tag="t1")
    nc.vector.tensor_scalar(
        out=t1[:], in0=n2[:], scalar1=1.0, scalar2=None, op0=ALU.add
    )
    den = pool.tile([np_, C], F32, tag="den")
    nc.vector.scalar_tensor_tensor(
        out=den[:], in0=nrm[:], scalar=EPS, in1=t1[:],
        op0=ALU.add, op1=ALU.mult,
    )
    rden = pool.tile([np_, C], F32, tag="rden")
    nc.vector.reciprocal(rden[:], den[:])
    sc = pool.tile([np_, C], F32, tag="sc")
    nc.vector.scalar_tensor_tensor(
        out=sc[:], in0=n2[:], scalar=pre, in1=rden[:],
        op0=ALU.mult, op1=ALU.mult,
    )
    nc.vector.tensor_tensor(
        out=v_ap.rearrange("p (c o) -> p c o", c=C),
        in0=s_ap.rearrange("p (c o) -> p c o", c=C),
        in1=sc[:].unsqueeze(2).broadcast_to([np_, C, OC]),
        op=ALU.mult,
    )


def _body(ctx, tc, xbd_d, wt_d, xp_d, sel_d, msk_d, out_d, K):
    nc = tc.nc

    per = ctx.enter_context(tc.tile_pool(name="per", bufs=1))
    xbdp = ctx.enter_context(tc.tile_pool(name="xbdp", bufs=4))
    wtp = ctx.enter_context(tc.tile_pool(name="wtp", bufs=4))
    xpp = ctx.enter_context(tc.tile_pool(name="xpp", bufs=4))
    upsum = ctx.enter_context(tc.tile_pool(name="upsum", bufs=6, space="PSUM"))
    spsum = ctx.enter_context(tc.tile_pool(name="spsum", bufs=1, space="PSUM"))
    vbpsum = ctx.enter_context(tc.tile_pool(name="vbpsum", bufs=1, space="PSUM"))
    tmpp = ctx.enter_context(tc.tile_pool(name="tmpp", bufs=2))
    small = ctx.enter_context(tc.tile_pool(name="small", bufs=3))

    # persistent SBUF state
    u1 = per.tile([128, NBG * K * CO], BF16)  # resident u
    u1v = u1[:].rearrange("p (k b x) -> p k b x", k=K, b=NBG)
    logits = per.tile([128, NBG * K * C], F32)
    logv = logits[:].rearrange("p (b k c) -> p b k c", b=NBG, k=K)
    cexp = per.tile([128, NBG * K * C], BF16)
    cexpv = cexp[:].rearrange("p (b k c) -> p b k c", b=NBG, k=K)
    cbdp = ctx.enter_context(tc.tile_pool(name="cbdp", bufs=2))
    sel_t = per.tile([8, 128], BF16)
    msk_t = per.tile([128, 8], BF16)
    s_sb = per.tile([BL, CO], F32)
    vb_a = per.tile([128, NBG * CO], BF16, tag="vb_a")
    vb_b = per.tile([128, NBG * CO], BF16, tag="vb_b")
    vb_ab = [vb_a, vb_b]

    nc.sync.dma_start(out=sel_t[:], in_=sel_d)
    nc.sync.dma_start(out=msk_t[:], in_=msk_d)

    # ---------------- u-phase ----------------
    # DMAs batched KB chunks at a time (DMA-issue cost dominates per-chunk
    # transfers); xbd on the sync queue, wt/xp on the gpsimd queue.
    # Small leading groups shorten the ramp (first matmul waits only a
    # 2-chunk transfer, not a full 8-chunk one); KB=8 amortizes DMA issue
    # cost in steady state. Tiles are all padded to the max group size.
    if K % 8 == 0:
        groups = [1, 1, 2, 4] + [8] * ((K - 8) // 8)
    elif K % 4 == 0:
        groups = [4] * (K // 4)
    else:
        groups = [1] * K
    assert sum(groups) == K
    KBMAX = max(groups)
    s1ps = spsum.tile([BL, CO], F32, tag="sps")
    k0 = 0
    for KB in groups:
        xbd_t = xbdp.tile([128, KBMAX * 512], BF16, tag="xbd")
        nc.sync.dma_start(
            out=xbd_t[:, : KB * 512].rearrange("p (k x) -> p k x", k=KB),
            in_=xbd_d[k0 : k0 + KB].rearrange("k p x -> p k x"),
        )
        wt_t = wtp.tile([128, KBMAX * CO], BF16, tag="wt")
        nc.gpsimd.dma_start(
            out=wt_t[:, : KB * CO].rearrange("p (k x) -> p k x", k=KB),
            in_=wt_d[k0 : k0 + KB].rearrange("k p x -> p k x"),
        )
        xp_t = xpp.tile([128, KBMAX * BL], BF16, tag="xp")
        nc.gpsimd.dma_start(
            out=xp_t[:, : KB * BL].rearrange("p (k x) -> p k x", k=KB),
            in_=xp_d[k0 : k0 + KB].rearrange("k p x -> p k x"),
        )
        for kk in range(KB):
            k = k0 + kk
            # iter-1 shortcut: accumulate sum_r u directly
            nc.tensor.matmul(
                s1ps[:],
                lhsT=xp_t[:, kk * BL : (kk + 1) * BL],
                rhs=wt_t[:, kk * CO : (kk + 1) * CO],
                start=(k == 0),
                stop=(k == K - 1),
            )
            for pair in range(2):
                ups = upsum.tile([128, 2 * CO], F32, tag="ups")
                for h in range(2):
                    bg = 2 * pair + h
                    nc.tensor.matmul(
                        ups[:, h * CO : (h + 1) * CO],
                        lhsT=xbd_t[:, kk * 512 + bg * 128 : kk * 512 + (bg + 1) * 128],
                        rhs=wt_t[:, kk * CO : (kk + 1) * CO],
                        start=True,
                        stop=True,
                    )
                dst = u1v[:, k, 2 * pair : 2 * pair + 2]
                src = ups[:].rearrange("p (h x) -> p h x", h=2)
                if pair == 0:
                    nc.scalar.copy(dst, src)
                else:
                    nc.vector.tensor_copy(out=dst, in_=src)
        k0 += KB

    def per_bg_v(bg, s_bg_ap, pre, vb_dst):
        """squash s (at partitions 0..7) -> v, broadcast into vb_dst[:, bg]."""
        v_bg = small.tile([8, CO], F32, tag="v_bg")
        _squash(nc, small, s_bg_ap, v_bg[:], pre=pre)
        vbf_bg = small.tile([8, CO], BF16, tag="vbf_bg")
        nc.scalar.copy(vbf_bg[:], v_bg[:])
        vbp = vbpsum.tile([128, CO], F32, tag="vbp")
        nc.tensor.matmul(vbp[:], lhsT=sel_t[:], rhs=vbf_bg[:], start=True, stop=True)
        nc.scalar.copy(vb_dst[:, bg * CO : (bg + 1) * CO], vbp[:])
        return v_bg

    # ---------------- iteration 1 ----------------
    nc.scalar.copy(s_sb[:], s1ps[:])
    for bg in range(NBG):
        s_bg = small.tile([8, CO], F32, tag="s_bg")
        nc.sync.dma_start(out=s_bg[:], in_=s_sb[bg * 8 : (bg + 1) * 8, :])
        per_bg_v(bg, s_bg[:], 1.0 / C, vb_ab[0])

    # ---------------- iterations 2..3 ----------------
    pending = None
    for it in (2, 3):
        vb = vb_ab[it % 2]
        vb_next = vb_ab[(it + 1) % 2]
        KH = K // 2
        for bg in range(NBG):
            sps = spsum.tile([80, CO], F32, tag="sps")
            for kh in range(2):
                ks = kh * KH
                # delta[b,r,c] = sum_o u*v  (vector engine)
                # delta[b,r,c] = sum_o u*v (vector engine; mul at 2x,
                # strided reduce at 1x -- splitting the reduce into
                # contiguous-halves trees measured WORSE: runs shorter
                # than ~16 elements drop the DVE below 1x)
                tmpt = tmpp.tile([128, KH * CO], BF16, tag="tmp")
                nc.vector.tensor_tensor(
                    out=tmpt[:].rearrange("p (k x) -> p k x", k=KH),
                    in0=u1v[:, ks : ks + KH, bg],
                    in1=vb[:, bg * CO : (bg + 1) * CO]
                    .unsqueeze(1)
                    .broadcast_to([128, KH, CO]),
                    op=ALU.mult,
                )
                red_in = tmpt[:].rearrange("p (k c o) -> p k c o", k=KH, c=C)
                lh = logv[:, bg, ks : ks + KH]
                if it == 2:
                    nc.vector.reduce_sum(out=lh, in_=red_in, axis=AX.X)
                else:
                    dtm = small.tile([128, KH * C], F32, tag="dtm")
                    nc.vector.reduce_sum(
                        out=dtm[:].rearrange("p (k c) -> p k c", k=KH),
                        in_=red_in,
                        axis=AX.X,
                    )
                    nc.vector.tensor_tensor(
                        out=lh.rearrange("p k c -> p (k c)"),
                        in0=lh.rearrange("p k c -> p (k c)"),
                        in1=dtm[:],
                        op=ALU.add,
                    )
                ch = cexpv[:, bg, ks : ks + KH]
                # softmax over c (no max subtraction; fp32 exp is safe here)
                nc.scalar.activation(ch, lh, AF.Exp)
                sume = small.tile([128, KH], F32, tag="sume")
                nc.vector.reduce_sum(out=sume[:], in_=ch, axis=AX.X)
                rs = small.tile([128, KH], F32, tag="rs")
                nc.vector.reciprocal(rs[:], sume[:])
                rsb = small.tile([128, KH], BF16, tag="rsb")
                nc.scalar.copy(rsb[:], rs[:])
                nc.gpsimd.tensor_tensor(
                    out=ch,
                    in0=ch,
                    in1=rsb[:].unsqueeze(2).broadcast_to([128, KH, C]),
                    op=ALU.mult,
                )
                # build block-diag c = c (x) delta-mask (zeros included)
                cbd_t = cbdp.tile([128, KH * C * 8], BF16, tag="cbd")
                cbdv = cbd_t[:].rearrange("p (k c e) -> p k c e", k=KH, c=C)
                # last bg: build on the vector engine to shorten the
                # iteration tail (gpsimd is ~3x slower per element)
                ceng = nc.vector if bg == NBG - 1 else nc.gpsimd
                ceng.tensor_tensor(
                    out=cbdv,
                    in0=ch.unsqueeze(3).broadcast_to([128, KH, C, 8]),
                    in1=msk_t[:]
                    .unsqueeze(1)
                    .unsqueeze(1)
                    .broadcast_to([128, KH, C, 8]),
                    op=ALU.mult,
                )
                # s-pass: PSUM-accumulated matmuls over the half's chunks
                for kk in range(KH):
                    nc.tensor.matmul(
                        sps[:],
                        lhsT=cbdv[:, kk].rearrange("p c e -> p (c e)"),
                        rhs=u1v[:, ks + kk, bg],
                        start=(ks + kk == 0),
                        stop=(ks + kk == K - 1),
                    )
            stmp = small.tile([80, CO], F32, tag="stmp")
            nc.scalar.copy(stmp[:], sps[:])
            # diagonal extract (c==c') via DMA, one [8,16] block per c
            s_bg2 = small.tile([8, CO], F32, tag="s_bg2")
            qs = (nc.sync, nc.gpsimd, nc.scalar)
            for c in range(C):
                qs[c % 3].dma_start(
                    out=s_bg2[:, c * OC : (c + 1) * OC],
                    in_=stmp[c * 8 : (c + 1) * 8, c * OC : (c + 1) * OC],
                )
            # Defer this group's squash/v-broadcast by one group: engine
            # queues are strict in-order, so emitting it here would
            # head-of-line-block the vector engine on the PE s-pass tail
            # before the next group's delta work could start.
            if pending is not None:
                pending()
            if it == 2:
                pending = lambda bg=bg, s=s_bg2, vn=vb_next: per_bg_v(
                    bg, s[:], 1.0, vn
                )
            else:
                # final iteration: land all groups in one [32,160] tile and
                # squash once at full width (per-group pipelining buys
                # nothing for the terminal output, and 8-partition squash
                # ops are pure overhead)
                def pending(bg=bg, s=s_bg2):
                    nc.sync.dma_start(
                        out=s_sb[bg * 8 : (bg + 1) * 8, :], in_=s[:]
                    )

    if pending is not None:
        pending()
    v_out = small.tile([BL, CO], F32, tag="v_out")
    _squash(nc, small, s_sb[:], v_out[:], pre=1.0, np_=BL)
    nc.sync.dma_start(out=out_d, in_=v_out[:])


def build(r=None):
    """Build and compile the Bass program. Returns the compiled Bacc."""
    K = _nchunks(r)
    nc = bacc.Bacc(
        "TRN2", target_bir_lowering=False, debug=False, num_devices=NCORES
    )
    xbd_d = nc.dram_tensor("xbd", [K, 128, 512], BF16, kind="ExternalInput").ap()
    wt_d = nc.dram_tensor("wt", [K, 128, CO], BF16, kind="ExternalInput").ap()
    xp_d = nc.dram_tensor("xp", [K, 128, BL], BF16, kind="ExternalInput").ap()
    sel_d = nc.dram_tensor("sel", [8, 128], BF16, kind="ExternalInput").ap()
    msk_d = nc.dram_tensor("msk", [128, 8], BF16, kind="ExternalInput").ap()
    out_d = nc.dram_tensor("v_out", [BL, CO], F32, kind="ExternalOutput").ap()
    with tile.TileContext(nc) as tc, ExitStack() as ctx:
        _body(ctx, tc, xbd_d, wt_d, xp_d, sel_d, msk_d, out_d, K)
    nc.compile()
    return nc


def make_inputs(x, weights, r=None):
    """Host-side marshalling: shard x over cores, rearrange to bf16 tiles."""
    K = _nchunks(r)
    r_full = K * G
    W = np.asarray(weights, dtype=np.float32)[0][:r_full]  # [R, C, IC, OC]
    wt = (
        W.reshape(K, G, C, IC, OC)
        .transpose(0, 1, 3, 2, 4)
        .reshape(K, 128, CO)
        .astype(NPBF)
    )
    sel = np.zeros((8, 128), dtype=np.float32)
    bi = np.arange(8)
    gi = np.arange(G)
    sel[bi[:, None], bi[:, None] * G + gi[None, :]] = 1.0
    sel = sel.astype(NPBF)
    msk = np.zeros((128, 8), dtype=np.float32)
    pi = np.arange(128)
    msk[pi, pi // G] = 1.0
    msk = msk.astype(NPBF)

    in_maps = []
    xf = np.asarray(x, dtype=np.float32)[:, :r_full]
    for core in range(NCORES):
        xl = xf[core * BL : (core + 1) * BL]  # [BL, R, IC]
        xr = xl.transpose(1, 2, 0).reshape(K, G, IC, BL)  # [K, g, i, b]
        xp = xr.reshape(K, 128, BL).astype(NPBF)
        xrg = xr.reshape(K, G, IC, NBG, 8)
        xbd6 = np.zeros((K, G, IC, NBG, 8, G), dtype=np.float32)
        for g in range(G):
            xbd6[:, g, :, :, :, g] = xrg[:, g]
        xbd = xbd6.reshape(K, 128, 512).astype(NPBF)
        in_maps.append(
            {"xbd": xbd, "wt": wt, "xp": xp, "sel": sel, "msk": msk}
        )
    return in_maps


_CACHE = {}


def kernel(x, weights):
    if "nc" not in _CACHE:
        _CACHE["nc"] = build()
    nc = _CACHE["nc"]
    in_maps = make_inputs(x, weights)
    res = run_bass_kernel_spmd(nc, in_maps, core_ids=list(range(NCORES)))
    outs = [res.results[i]["v_out"].reshape(BL, C, OC) for i in range(NCORES)]
    return np.concatenate(outs, axis=0)


# revision 38
# speedup vs baseline: 1.0240x; 1.0240x over previous
"""DigitCaps dynamic-routing kernel for Trainium2 (8 NeuronCores, SPMD).

Problem:  u = einsum('bri,rcio->brco', x, W[0]);  3 routing iterations
          (softmax over capsules, weighted sum over routes, squash,
          agreement update);  returns v [B, C, OC].

Shapes: B=256, R=1152, C=10, IC=8, OC=16.  Batch-sharded 8 ways (BL=32
per core, zero cross-core communication).

Design notes (per core):
 - u-phase: r is processed in 72 chunks of G=16 routes.  For each chunk,
   lhsT is a block-diagonal arrangement of x ([128=(g,i), 128=(b8,g16)]
   per b-group of 8) so a single matmul produces u for 8 batches x 16
   routes = 128 PSUM partitions; rhs is the natural W chunk
   [128=(g,i), 160=(c,o)].  bf16 operands, fp32 PSUM accumulate.
 - u stays resident in SBUF as bf16 [p=(b8,g16), f=(bg4, k72, c10, o16)].
 - Iteration 1 shortcut: c is uniform 1/10, so s = 0.1*sum_r u, computed
   for free during the u-phase by one extra accumulating matmul per chunk
   with plain (non-block-diag) x as lhsT.
 - b-update (delta = sum_o u*v) runs on the vector engine (mul at 2x +
   strided reduce at 1x: this is the kernel's port-bound critical path);
   softmax on scalar+vector engines (max-subtraction skipped: logits are
   O(30), fp32 exp is safe and matches jax softmax up to rounding).
 - s-pass (s = sum_r c*u) runs on the tensor engine: lhsT is block-diag
   c [(b,g), (c10,b'8)] built by a gpsimd mask-multiply, rhs is the
   resident u slice, PSUM-accumulated over all 72 chunks; the (c,c')
   diagonal is then extracted with small DMAs (engine APs cannot express
   partition-crossing diagonals, DMAs can move between partition bases).
 - Everything in the routing iterations is processed per 8-batch group
   (and per 36-chunk half) so the DVE chain of one group overlaps the
   gpsimd/PE/DMA tail of the previous one, including across iterations
   (per-group squash + v-broadcast feeds the next iteration early).
"""

import sys

sys.path.insert(0, "/opt/trn_rl_repo")

from contextlib import ExitStack

import ml_dtypes
import numpy as np

import concourse.bass as bass
import concourse.tile as tile
from concourse import bacc, mybir
from concourse.bass_utils import run_bass_kernel_spmd

BF16 = mybir.dt.bfloat16
F32 = mybir.dt.float32
AF = mybir.ActivationFunctionType
ALU = mybir.AluOpType
AX = mybir.AxisListType

B, R, C, IC, OC = 256, 1152, 10, 8, 16
NCORES = 8
BL = B // NCORES  # 32 batches per core
G = 16  # routes per chunk
NBG = BL // 8  # 4 b-groups of 8
CO = C * OC  # 160
EPS = 1e-8
NPBF = ml_dtypes.bfloat16

# Set by tests to shrink the problem for simulation; full size by default.
_R_OVERRIDE = None


def _nchunks(r=None):
    r = r if r is not None else (_R_OVERRIDE or R)
    assert r % G == 0
    return r // G


def _squash(nc, pool, s_ap, v_ap, pre, np_=8):
    """v = squash(pre*s) for s,v [np_, CO] fp32 SBUF tiles (base partition 0).

    Uses sqrt(n2 + EPS^2) ~= nrm + EPS (difference is O(EPS) absolute,
    only relevant when the norm itself is ~EPS).
    """
    sq = pool.tile([np_, CO], F32, tag="sq")
    if pre == 1.0:
        nc.vector.tensor_tensor(out=sq[:], in0=s_ap, in1=s_ap, op=ALU.mult)
    else:
        nc.scalar.activation(sq[:], s_ap, AF.Square, scale=pre)  # (pre*s)^2
    n2 = pool.tile([np_, C], F32, tag="n2")
    nc.vector.reduce_sum(
        out=n2[:], in_=sq[:].rearrange("p (c o) -> p c o", c=C), axis=AX.X
    )
    nrm = pool.tile([np_, C], F32, tag="nrm")
    nc.scalar.sqrt(nrm[:], n2[:])
    t1 = pool.tile([np_, C], F32, tag="t1")
    nc.vector.tensor_scalar(
        out=t1[:], in0=n2[:], scalar1=1.0, scalar2=None, op0=ALU.add
    )
    den = pool.tile([np_, C], F32, tag="den")
    nc.vector.scalar_tensor_tensor(
        out=den[:], in0=nrm[:], scalar=EPS, in1=t1[:],
        op0=ALU.add, op1=ALU.mult,
    )
    rden = pool.tile([np_, C], F32, tag="rden")
    nc.vector.reciprocal(rden[:], den[:])
    sc = pool.tile([np_, C], F32, tag="sc")
    nc.vector.scalar_tensor_tensor(
        out=sc[:], in0=n2[:], scalar=pre, in1=rden[:],
        op0=ALU.mult, op1=ALU.mult,
    )
    nc.vector.tensor_tensor(
        out=v_ap.rearrange("p (c o) -> p c o", c=C),
        in0=s_ap.rearrange("p (c o) -> p c o", c=C),
        in1=sc[:].unsqueeze(2).broadcast_to([np_, C, OC]),
        op=ALU.mult,
    )


def _body(ctx, tc, xbd_d, wt_d, xp_d, sel_d, msk_d, out_d, K):
    nc = tc.nc

    per = ctx.enter_context(tc.tile_pool(name="per", bufs=1))
    xbdp = ctx.enter_context(tc.tile_pool(name="xbdp", bufs=4))
    wtp = ctx.enter_context(tc.tile_pool(name="wtp", bufs=4))
    xpp = ctx.enter_context(tc.tile_pool(name="xpp", bufs=4))
    upsum = ctx.enter_context(tc.tile_pool(name="upsum", bufs=6, space="PSUM"))
    spsum = ctx.enter_context(tc.tile_pool(name="spsum", bufs=1, space="PSUM"))
    vbpsum = ctx.enter_context(tc.tile_pool(name="vbpsum", bufs=1, space="PSUM"))
    tmpp = ctx.enter_context(tc.tile_pool(name="tmpp", bufs=2))
    small = ctx.enter_context(tc.tile_pool(name="small", bufs=3))

    # persistent SBUF state
    u1 = per.tile([128, NBG * K * CO], BF16)  # resident u
    u1v = u1[:].rearrange("p (k b x) -> p k b x", k=K, b=NBG)
    logits = per.tile([128, NBG * K * C], F32)
    logv = logits[:].rearrange("p (b k c) -> p b k c", b=NBG, k=K)
    cexp = per.tile([128, NBG * K * C], BF16)
    cexpv = cexp[:].rearrange("p (b k c) -> p b k c", b=NBG, k=K)
    cbdp = ctx.enter_context(tc.tile_pool(name="cbdp", bufs=2))
    sel_t = per.tile([8, 128], BF16)
    msk_t = per.tile([128, 8], BF16)
    s_sb = per.tile([BL, CO], F32)
    vb_a = per.tile([128, NBG * CO], BF16, tag="vb_a")
    vb_b = per.tile([128, NBG * CO], BF16, tag="vb_b")
    vb_ab = [vb_a, vb_b]

    nc.sync.dma_start(out=sel_t[:], in_=sel_d)
    nc.sync.dma_start(out=msk_t[:], in_=msk_d)

    # ---------------- u-phase ----------------
    # DMAs batched KB chunks at a time (DMA-issue cost dominates per-chunk
    # transfers); xbd on the sync queue, wt/xp on the gpsimd queue.
    # Small leading groups shorten the ramp (first matmul waits only a
    # 2-chunk transfer, not a full 8-chunk one); KB=8 amortizes DMA issue
    # cost in steady state. Tiles are all padded to the max group size.
    if K % 8 == 0:
        groups = [2, 2, 4] + [8] * ((K - 8) // 8)
    elif K % 4 == 0:
        groups = [4] * (K // 4)
    else:
        groups = [1] * K
    assert sum(groups) == K
    KBMAX = max(groups)
    s1ps = spsum.tile([BL, CO], F32, tag="sps")
    k0 = 0
    for KB in groups:
        xbd_t = xbdp.tile([128, KBMAX * 512], BF16, tag="xbd")
        nc.sync.dma_start(
            out=xbd_t[:, : KB * 512].rearrange("p (k x) -> p k x", k=KB),
            in_=xbd_d[k0 : k0 + KB].rearrange("k p x -> p k x"),
        )
        wt_t = wtp.tile([128, KBMAX * CO], BF16, tag="wt")
        nc.gpsimd.dma_start(
            out=wt_t[:, : KB * CO].rearrange("p (k x) -> p k x", k=KB),
            in_=wt_d[k0 : k0 + KB].rearrange("k p x -> p k x"),
        )
        xp_t = xpp.tile([128, KBMAX * BL], BF16, tag="xp")
        nc.gpsimd.dma_start(
            out=xp_t[:, : KB * BL].rearrange("p (k x) -> p k x", k=KB),
            in_=xp_d[k0 : k0 + KB].rearrange("k p x -> p k x"),
        )
        for kk in range(KB):
            k = k0 + kk
            # iter-1 shortcut: accumulate sum_r u directly
            nc.tensor.matmul(
                s1ps[:],
                lhsT=xp_t[:, kk * BL : (kk + 1) * BL],
                rhs=wt_t[:, kk * CO : (kk + 1) * CO],
                start=(k == 0),
                stop=(k == K - 1),
            )
            for pair in range(2):
                ups = upsum.tile([128, 2 * CO], F32, tag="ups")
                for h in range(2):
                    bg = 2 * pair + h
                    nc.tensor.matmul(
                        ups[:, h * CO : (h + 1) * CO],
                        lhsT=xbd_t[:, kk * 512 + bg * 128 : kk * 512 + (bg + 1) * 128],
                        rhs=wt_t[:, kk * CO : (kk + 1) * CO],
                        start=True,
                        stop=True,
                    )
                dst = u1v[:, k, 2 * pair : 2 * pair + 2]
                src = ups[:].rearrange("p (h x) -> p h x", h=2)
                if pair == 0:
                    nc.scalar.copy(dst, src)
                else:
                    nc.vector.tensor_copy(out=dst, in_=src)
        k0 += KB

    def per_bg_v(bg, s_bg_ap, pre, vb_dst):
        """squash s (at partitions 0..7) -> v, broadcast into vb_dst[:, bg]."""
        v_bg = small.tile([8, CO], F32, tag="v_bg")
        _squash(nc, small, s_bg_ap, v_bg[:], pre=pre)
        vbf_bg = small.tile([8, CO], BF16, tag="vbf_bg")
        nc.scalar.copy(vbf_bg[:], v_bg[:])
        vbp = vbpsum.tile([128, CO], F32, tag="vbp")
        nc.tensor.matmul(vbp[:], lhsT=sel_t[:], rhs=vbf_bg[:], start=True, stop=True)
        nc.scalar.copy(vb_dst[:, bg * CO : (bg + 1) * CO], vbp[:])
        return v_bg

    # ---------------- iteration 1 ----------------
    nc.scalar.copy(s_sb[:], s1ps[:])
    for bg in range(NBG):
        s_bg = small.tile([8, CO], F32, tag="s_bg")
        nc.sync.dma_start(out=s_bg[:], in_=s_sb[bg * 8 : (bg + 1) * 8, :])
        per_bg_v(bg, s_bg[:], 1.0 / C, vb_ab[0])

    # ---------------- iterations 2..3 ----------------
    pending = None
    for it in (2, 3):
        vb = vb_ab[it % 2]
        vb_next = vb_ab[(it + 1) % 2]
        KH = K // 2
        for bg in range(NBG):
            sps = spsum.tile([80, CO], F32, tag="sps")
            for kh in range(2):
                ks = kh * KH
                # delta[b,r,c] = sum_o u*v  (vector engine)
                # delta[b,r,c] = sum_o u*v (vector engine; mul at 2x,
                # strided reduce at 1x -- splitting the reduce into
                # contiguous-halves trees measured WORSE: runs shorter
                # than ~16 elements drop the DVE below 1x)
                tmpt = tmpp.tile([128, KH * CO], BF16, tag="tmp")
                nc.vector.tensor_tensor(
                    out=tmpt[:].rearrange("p (k x) -> p k x", k=KH),
                    in0=u1v[:, ks : ks + KH, bg],
                    in1=vb[:, bg * CO : (bg + 1) * CO]
                    .unsqueeze(1)
                    .broadcast_to([128, KH, CO]),
                    op=ALU.mult,
                )
                red_in = tmpt[:].rearrange("p (k c o) -> p k c o", k=KH, c=C)
                lh = logv[:, bg, ks : ks + KH]
                if it == 2:
                    nc.vector.reduce_sum(out=lh, in_=red_in, axis=AX.X)
                else:
                    dtm = small.tile([128, KH * C], F32, tag="dtm")
                    nc.vector.reduce_sum(
                        out=dtm[:].rearrange("p (k c) -> p k c", k=KH),
                        in_=red_in,
                        axis=AX.X,
                    )
                    nc.vector.tensor_tensor(
                        out=lh.rearrange("p k c -> p (k c)"),
                        in0=lh.rearrange("p k c -> p (k c)"),
                        in1=dtm[:],
                        op=ALU.add,
                    )
                ch = cexpv[:, bg, ks : ks + KH]
                # softmax over c (no max subtraction; fp32 exp is safe here)
                nc.scalar.activation(ch, lh, AF.Exp)
                sume = small.tile([128, KH], F32, tag="sume")
                nc.vector.reduce_sum(out=sume[:], in_=ch, axis=AX.X)
                rs = small.tile([128, KH], F32, tag="rs")
                nc.vector.reciprocal(rs[:], sume[:])
                rsb = small.tile([128, KH], BF16, tag="rsb")
                nc.scalar.copy(rsb[:], rs[:])
                nc.gpsimd.tensor_tensor(
                    out=ch,
                    in0=ch,
                    in1=rsb[:].unsqueeze(2).broadcast_to([128, KH, C]),
                    op=ALU.mult,
                )
                # build block-diag c = c (x) delta-mask (zeros included)
                cbd_t = cbdp.tile([128, KH * C * 8], BF16, tag="cbd")
                cbdv = cbd_t[:].rearrange("p (k c e) -> p k c e", k=KH, c=C)
                # last bg: build on the vector engine to shorten the
                # iteration tail (gpsimd is ~3x slower per element)
                ceng = nc.vector if bg == NBG - 1 else nc.gpsimd
                ceng.tensor_tensor(
                    out=cbdv,
                    in0=ch.unsqueeze(3).broadcast_to([128, KH, C, 8]),
                    in1=msk_t[:]
                    .unsqueeze(1)
                    .unsqueeze(1)
                    .broadcast_to([128, KH, C, 8]),
                    op=ALU.mult,
                )
                # s-pass: PSUM-accumulated matmuls over the half's chunks
                for kk in range(KH):
                    nc.tensor.matmul(
                        sps[:],
                        lhsT=cbdv[:, kk].rearrange("p c e -> p (c e)"),
                        rhs=u1v[:, ks + kk, bg],
                        start=(ks + kk == 0),
                        stop=(ks + kk == K - 1),
                    )
            stmp = small.tile([80, CO], F32, tag="stmp")
            nc.scalar.copy(stmp[:], sps[:])
            # diagonal extract (c==c') via DMA, one [8,16] block per c
            s_bg2 = small.tile([8, CO], F32, tag="s_bg2")
            qs = (nc.sync, nc.gpsimd, nc.scalar)
            for c in range(C):
                qs[c % 3].dma_start(
                    out=s_bg2[:, c * OC : (c + 1) * OC],
                    in_=stmp[c * 8 : (c + 1) * 8, c * OC : (c + 1) * OC],
                )
            # Defer this group's squash/v-broadcast by one group: engine
            # queues are strict in-order, so emitting it here would
            # head-of-line-block the vector engine on the PE s-pass tail
            # before the next group's delta work could start.
            if pending is not None:
                pending()
            if it == 2:
                pending = lambda bg=bg, s=s_bg2, vn=vb_next: per_bg_v(
                    bg, s[:], 1.0, vn
                )
            else:
                # final iteration: land all groups in one [32,160] tile and
                # squash once at full width (per-group pipelining buys
                # nothing for the terminal output, and 8-partition squash
                # ops are pure overhead)
                def pending(bg=bg, s=s_bg2):
                    nc.sync.dma_start(
                        out=s_sb[bg * 8 : (bg + 1) * 8, :], in_=s[:]
                    )

    if pending is not None:
        pending()
    v_out = small.tile([BL, CO], F32, tag="v_out")
    _squash(nc, small, s_sb[:], v_out[:], pre=1.0, np_=BL)
    nc.sync.dma_start(out=out_d, in_=v_out[:])


def build(r=None):
    """Build and compile the Bass program. Returns the compiled Bacc."""
    K = _nchunks(r)
    nc = bacc.Bacc(
        "TRN2", target_bir_lowering=False, debug=False, num_devices=NCORES
    )
    xbd_d = nc.dram_tensor("xbd", [K, 128, 512], BF16, kind="ExternalInput").ap()
    wt_d = nc.dram_tensor("wt", [K, 128, CO], BF16, kind="ExternalInput").ap()
    xp_d = nc.dram_tensor("xp", [K, 128, BL], BF16, kind="ExternalInput").ap()
    sel_d = nc.dram_tensor("sel", [8, 128], BF16, kind="ExternalInput").ap()
    msk_d = nc.dram_tensor("msk", [128, 8], BF16, kind="ExternalInput").ap()
    out_d = nc.dram_tensor("v_out", [BL, CO], F32, kind="ExternalOutput").ap()
    with tile.TileContext(nc) as tc, ExitStack() as ctx:
        _body(ctx, tc, xbd_d, wt_d, xp_d, sel_d, msk_d, out_d, K)
    nc.compile()
    return nc


def make_inputs(x, weights, r=None):
    """Host-side marshalling: shard x over cores, rearrange to bf16 tiles."""
    K = _nchunks(r)
    r_full = K * G
    W = np.asarray(weights, dtype=np.float32)[0][:r_full]  # [R, C, IC, OC]
    wt = (
        W.reshape(K, G, C, IC, OC)
        .transpose(0, 1, 3, 2, 4)
        .reshape(K, 128, CO)
        .astype(NPBF)
    )
    sel = np.zeros((8, 128), dtype=np.float32)
    bi = np.arange(8)
    gi = np.arange(G)
    sel[bi[:, None], bi[:, None] * G + gi[None, :]] = 1.0
    sel = sel.astype(NPBF)
    msk = np.zeros((128, 8), dtype=np.float32)
    pi = np.arange(128)
    msk[pi, pi // G] = 1.0
    msk = msk.astype(NPBF)

    in_maps = []
    xf = np.asarray(x, dtype=np.float32)[:, :r_full]
    for core in range(NCORES):
        xl = xf[core * BL : (core + 1) * BL]  # [BL, R, IC]
        xr = xl.transpose(1, 2, 0).reshape(K, G, IC, BL)  # [K, g, i, b]
        xp = xr.reshape(K, 128, BL).astype(NPBF)
        xrg = xr.reshape(K, G, IC, NBG, 8)
        xbd6 = np.zeros((K, G, IC, NBG, 8, G), dtype=np.float32)
        for g in range(G):
            xbd6[:, g, :, :, :, g] = xrg[:, g]
        xbd = xbd6.reshape(K, 128, 512).astype(NPBF)
        in_maps.append(
            {"xbd": xbd, "wt": wt, "xp": xp, "sel": sel, "msk": msk}
        )
    return in_maps


_CACHE = {}


def kernel(x, weights):
    if "nc" not in _CACHE:
        _CACHE["nc"] = build()
    nc = _CACHE["nc"]
    in_maps = make_inputs(x, weights)
    res = run_bass_kernel_spmd(nc, in_maps, core_ids=list(range(NCORES)))
    outs = [res.results[i]["v_out"].reshape(BL, C, OC) for i in range(NCORES)]
    return np.concatenate(outs, axis=0)
